# revision 3
# baseline (speedup 1.0000x reference)
"""Trainium2 Bass kernel for Conv2D(sum of 20 1x1 convs) + QwenRMSNorm.

Math: y = einsum("bsi,loi->bso", x, conv_w) / L ; out = rmsnorm(y) * norm_w.
Since x does not depend on l, the 20-matrix contraction collapses to a single
matmul with W = sum_l conv_w[l] / L.  Host pre-sums/transposes/casts the weight
(one [H,H] matrix bf16) and lays out x as token-sharded, hidden-major bf16
slabs; the 8 NeuronCores each run matmul (bf16, fp32 accum) + RMSNorm on their
2048 tokens.  All device compute is token-local; no collectives.

v3 schedule (trace-driven rework of v2):
 - MMs are ib-outer over GROUPS of 2 token tiles (4 PSUM bufs = all 8 banks),
   so real matmuls start as soon as x(tile0,1 lo-half) + w[ib0] land (~9.7us)
   instead of waiting for the whole 2MB weight fill (~16.9us in the v2 trace).
 - Input DMA descriptors are issued on the sync HWDGE ring in exact
   consumption order: x0/x1 lo, w0..w4, x0/x1 hi, w5..w7, x bulk.  One
   descriptor is >=128KB so the ~650ns/issue engine cadence never starves
   the ~390GB/s queue.
 - Warm-up MMs are N=128 on a small memset tile (fast cold dispatch) so the
   PE HAM clock-gate opens ~when the first real MM is data-ready.
 - Norm chain per tile: ACT square+accum -> ACT sqrt -> DVE recip, then the
   two output halves are scaled in PARALLEL (ACT Copy*scale on half1, DVE
   tensor_scalar on half0) to cut chain latency; one 256KB out DMA per tile,
   alternating scalar/gpsimd rings.
 - LAST tile runs oh1-first so its half1 square + sqrt-bias prep happen under
   the oh0 MMs; after the last MM only square(half0) -> sqrt -> recip ->
   parallel scales -> two half DMAs (sync+scalar HWDGE) remain (~4.4us tail
   vs 5.9us in v2).
 - norm_w == 1 (the spec's fill) skips the norm_w multiply; a general variant
   handles arbitrary norm_w.
"""

import numpy as np
import ml_dtypes
from contextlib import ExitStack

import concourse.bass as bass
import concourse.mybir as mybir
import concourse.tile as tile
from concourse.bass_utils import run_bass_kernel_spmd

N_CORES = 8
B, S, H, L = 4, 4096, 1024, 20
TOK = B * S               # 16384 tokens
TPC = TOK // N_CORES      # 2048 tokens per core
TB = TPC // 128           # 16 token-blocks of 128 per core
KB = H // 128             # 8 contraction blocks
NOH = H // 512            # 2 psum halves of the output row
EPS = 1e-6
N_WARM = 26               # HAM warm-up matmuls (N=128) during the DMA fill

BF16 = mybir.dt.bfloat16
F32 = mybir.dt.float32
AF = mybir.ActivationFunctionType
OP = mybir.AluOpType

_BUILT = {}          # variant -> cached Bass program
LAST_RESULTS = None  # BassKernelResults of the most recent run (for test harness)


def _legalize_multiwait(nc):
    """The walrus build here encodes exactly one semaphore wait per 64B
    instruction (NEURON_ISA_TPB_EVENTS has a single wait slot) and errors on
    Tile's multi-wait instructions.  Split surplus waits into standalone
    EVENT_SEMAPHORE instructions on the same engine, placed directly before
    the original instruction (same sequencer stream -> same semantics)."""
    n_ev = 0
    for f in nc.m.functions:
        for blk in f.blocks:
            insts = blk.instructions
            out = []
            changed = False
            for inst in list(insts):
                si = getattr(inst, "sync_info", None)
                waits = list(si.on_wait) if si is not None else []
                if len(waits) > 1:
                    changed = True
                    updates = list(si.on_update)
                    for w in waits[:-1]:
                        ev = mybir.InstEventSemaphore(
                            name=f"{inst.name}-sw{n_ev}", ins=[], outs=[])
                        n_ev += 1
                        ev.engine = inst.engine
                        ev.sync_info = mybir.SyncInfo(on_wait=[w], on_update=[])
                        out.append(ev)
                    inst.sync_info = mybir.SyncInfo(
                        on_wait=[waits[-1]], on_update=updates)
                out.append(inst)
            if changed:
                insts.clear()
                insts.extend(out)


def _build(with_nw):
    nc = bass.Bass()
    # x layout (partition-outermost): xt[p, tt, ib, t] = x[tt*128+t, ib*128+p]
    xt_h = nc.dram_tensor("xt", [128, TB, KB, 128], BF16, kind="ExternalInput")
    # w layout: wt[p, ib, oh, j] = W[oh*512+j, ib*128+p], W = sum_l conv_w[l]/L
    wt_h = nc.dram_tensor("wt", [128, KB, NOH, 512], BF16,
                          kind="ExternalInput")
    if with_nw:
        nw_h = nc.dram_tensor("nw", [H], F32, kind="ExternalInput")
    out_h = nc.dram_tensor("out", [TPC, H], BF16, kind="ExternalOutput")

    with tile.TileContext(nc) as tc, ExitStack() as ctx:
        xpool = ctx.enter_context(tc.tile_pool(name="x", bufs=1))
        wpool = ctx.enter_context(tc.tile_pool(name="w", bufs=1))
        cpool = ctx.enter_context(tc.tile_pool(name="consts", bufs=1))
        opool = ctx.enter_context(tc.tile_pool(name="out", bufs=6))
        spool = ctx.enter_context(tc.tile_pool(name="scratch", bufs=2))
        stats = ctx.enter_context(tc.tile_pool(name="stats", bufs=8))
        psum = ctx.enter_context(tc.tile_pool(name="psum", bufs=4, space="PSUM"))

        # const tiles first so the warm-up matmuls' source is ready early
        wu_sb = cpool.tile([128, 128], BF16)
        nc.vector.memset(wu_sb, 0.0)
        zero_sb = cpool.tile([128, 1], F32)
        nc.vector.memset(zero_sb, 0.0)
        eps_sb = cpool.tile([128, 1], F32)
        nc.vector.memset(eps_sb, EPS)

        x_sb = xpool.tile([128, TB, KB, 128], BF16)
        w_sb = wpool.tile([128, KB, NOH, 512], BF16)

        # Input DMA, sync HWDGE ring, in exact consumption order.  Descriptors
        # are >=128KB so the per-issue engine cadence (~650ns) keeps the queue
        # saturated at the ~390GB/s large-transfer rate.
        nc.sync.dma_start(out=x_sb[:, 0, 0:4], in_=xt_h[:, 0, 0:4])
        nc.sync.dma_start(out=x_sb[:, 1, 0:4], in_=xt_h[:, 1, 0:4])
        for ib in range(5):                                   # w0..w4
            nc.sync.dma_start(out=w_sb[:, ib:ib + 1], in_=wt_h[:, ib:ib + 1])
        nc.sync.dma_start(out=x_sb[:, 0, 4:8], in_=xt_h[:, 0, 4:8])
        nc.sync.dma_start(out=x_sb[:, 1, 4:8], in_=xt_h[:, 1, 4:8])
        for ib in range(5, KB):                               # w5..w7
            nc.sync.dma_start(out=w_sb[:, ib:ib + 1], in_=wt_h[:, ib:ib + 1])
        nc.sync.dma_start(out=x_sb[:, 2:4], in_=xt_h[:, 2:4])
        nc.sync.dma_start(out=x_sb[:, 4:8], in_=xt_h[:, 4:8])
        nc.sync.dma_start(out=x_sb[:, 8:16], in_=xt_h[:, 8:16])

        if with_nw:
            # norm_w broadcast to 128 partitions (general path only)
            nw_sb = cpool.tile([128, H], F32)
            nc.gpsimd.dma_start(
                out=nw_sb,
                in_=bass.AP(tensor=nw_h, offset=0, ap=[[0, 128], [1, H]]))

        # preload the ACT function table (Square/Sqrt) during the fill
        dummy = stats.tile([128, 1], F32)
        nc.scalar.activation(out=dummy, in_=zero_sb, func=AF.Square,
                             bias=zero_sb)

        # HAM warm-up: small N=128 matmuls dispatch quickly even at the cold
        # 1.2GHz clock, keeping the PE busy from ~7us so the HAM clock gate
        # is at 8/8 about when the first real matmul's data lands.
        wp = psum.tile([128, H], F32, name="wp", tag="yp")
        for _ in range(N_WARM):
            nc.tensor.matmul(wp[:, 0:128], wu_sb, wu_sb, start=True, stop=True)

        def norm_chain(tt, yp, last=False):
            """RMSNorm + scaled output + out-DMA for one 128-token tile."""
            rows = slice(tt * 128, (tt + 1) * 128)
            o_sb = opool.tile([128, H], BF16)
            if last:
                # halves were squared separately (oh1 early, under oh0 MMs)
                sq1 = spool.tile([128, 512], BF16, tag="sq")
                ssum1 = stats.tile([128, 1], F32)
                nc.scalar.activation(out=sq1, in_=yp[:, 512:1024],
                                     func=AF.Square, bias=zero_sb,
                                     accum_out=ssum1)
                # bias-prep for the final sqrt: b = ssum1/H + eps
                bprep = stats.tile([128, 1], F32)
                nc.scalar.activation(out=bprep, in_=ssum1, func=AF.Identity,
                                     bias=eps_sb, scale=1.0 / H)
                yield  # emit oh0 MMs in the caller, then resume
                sq0 = spool.tile([128, 512], BF16, tag="sq")
                ssum0 = stats.tile([128, 1], F32)
                nc.scalar.activation(out=sq0, in_=yp[:, 0:512],
                                     func=AF.Square, bias=zero_sb,
                                     accum_out=ssum0)
                std = stats.tile([128, 1], F32)
                nc.scalar.activation(out=std, in_=ssum0, func=AF.Sqrt,
                                     bias=bprep, scale=1.0 / H)
            else:
                sq = spool.tile([128, H], BF16, tag="sq")
                ssum = stats.tile([128, 1], F32)
                nc.scalar.activation(out=sq, in_=yp, func=AF.Square,
                                     bias=zero_sb, accum_out=ssum)
                std = stats.tile([128, 1], F32)
                nc.scalar.activation(out=std, in_=ssum, func=AF.Sqrt,
                                     bias=eps_sb, scale=1.0 / H)
            rstd = stats.tile([128, 1], F32)
            nc.vector.reciprocal(out=rstd, in_=std)
            # scale the two halves in parallel: half1 on ACT, half0 on DVE
            if with_nw:
                nc.vector.scalar_tensor_tensor(
                    out=o_sb[:, 512:1024], in0=yp[:, 512:1024], scalar=rstd,
                    in1=nw_sb[:, 512:1024], op0=OP.mult, op1=OP.mult)
                nc.vector.scalar_tensor_tensor(
                    out=o_sb[:, 0:512], in0=yp[:, 0:512], scalar=rstd,
                    in1=nw_sb[:, 0:512], op0=OP.mult, op1=OP.mult)
            else:
                nc.scalar.activation(out=o_sb[:, 512:1024],
                                     in_=yp[:, 512:1024], func=AF.Copy,
                                     scale=rstd)
                nc.vector.tensor_scalar_mul(out=o_sb[:, 0:512],
                                            in0=yp[:, 0:512], scalar1=rstd)
            if last:
                # two half DMAs on the two idle HWDGE rings
                nc.scalar.dma_start(out=out_h[rows, 512:1024],
                                    in_=o_sb[:, 512:1024])
                nc.sync.dma_start(out=out_h[rows, 0:512],
                                  in_=o_sb[:, 0:512])
            else:
                # one 256KB contiguous DMA; alternate rings per tile.  SWDGE
                # (gpsimd) only for early tiles so its queue is long drained
                # by tile-context exit.
                eng = nc.gpsimd if (tt % 2 == 0 and tt < 13) else nc.scalar
                eng.dma_start(out=out_h[rows, :], in_=o_sb)

        # groups of 2 token tiles, ib-outer inside the group: matmuls start
        # once x(lo)+w[ib0] are resident and stay DMA-fed at ~300GB/s.
        groups = [(0, 1), (2, 3), (4, 5), (6, 7), (8, 9), (10, 11), (12, 13),
                  (14,), (15,)]
        for tts in groups:
            yps = [psum.tile([128, H], F32, name=f"yp{tt}", tag="yp")
                   for tt in tts]
            if tts[-1] == TB - 1:
                # last tile: oh1 MMs first so its square/bias-prep run under
                # the oh0 MMs; only the short half0 chain trails the last MM.
                tt = tts[0]
                yp = yps[0]
                gen = norm_chain(tt, yp, last=True)
                for oh in (1, 0):
                    for ib in range(KB):
                        nc.tensor.matmul(
                            yp[:, oh * 512:(oh + 1) * 512],
                            x_sb[:, tt, ib, :],
                            w_sb[:, ib, oh, :],
                            start=(ib == 0), stop=(ib == KB - 1))
                    if oh == 1:
                        next(gen)       # emit early half1 square + bias prep
                for _ in gen:           # emit the trailing chain
                    pass
            else:
                for ib in range(KB):
                    for i, tt in enumerate(tts):
                        for oh in range(NOH):
                            nc.tensor.matmul(
                                yps[i][:, oh * 512:(oh + 1) * 512],
                                x_sb[:, tt, ib, :],
                                w_sb[:, ib, oh, :],
                                start=(ib == 0), stop=(ib == KB - 1))
                for i, tt in enumerate(tts):
                    for _ in norm_chain(tt, yps[i]):
                        pass

    _legalize_multiwait(nc)
    return nc


def host_prep(x, conv_w, norm_w, with_nw):
    """Shard + lay out the full inputs into per-core device input maps."""
    bf16 = ml_dtypes.bfloat16

    # Collapse the 20 1x1 convs: W[o,i] = sum_l conv_w[l,o,i] / L
    w = np.asarray(conv_w).sum(axis=0) * (1.0 / L)          # [H(o), H(i)] f32
    # wt[p, ib, oh, j] = W[oh*512+j, ib*128+p]
    wt = np.ascontiguousarray(
        w.reshape(NOH, 512, KB, 128).transpose(3, 2, 0, 1).astype(bf16))

    x2d = np.asarray(x).reshape(TOK, H)
    xbf = x2d.astype(bf16)

    in_maps = []
    for c in range(N_CORES):
        xc = xbf[c * TPC:(c + 1) * TPC]                      # [TPC, H]
        # xt[p, tt, ib, t] = xc[tt*128+t, ib*128+p]
        xtc = np.ascontiguousarray(
            xc.reshape(TB, 128, KB, 128).transpose(3, 0, 2, 1))
        m = {"xt": xtc, "wt": wt}
        if with_nw:
            m["nw"] = np.ascontiguousarray(np.asarray(norm_w),
                                           dtype=np.float32)
        in_maps.append(m)
    return in_maps


def kernel(x, conv_w, norm_w):
    global LAST_RESULTS

    x = np.asarray(x)
    out_dtype = x.dtype
    nw = np.asarray(norm_w)
    with_nw = not bool(np.all(nw == 1.0))

    if with_nw not in _BUILT:
        _BUILT[with_nw] = _build(with_nw)
    nc = _BUILT[with_nw]

    in_maps = host_prep(x, conv_w, norm_w, with_nw)

    res = run_bass_kernel_spmd(nc, in_maps, core_ids=list(range(N_CORES)))
    LAST_RESULTS = res

    out = np.concatenate([r["out"] for r in res.results], axis=0)
    return out.reshape(B, S, H).astype(out_dtype, copy=False)


# revision 6
# speedup vs baseline: 1.1606x; 1.1606x over previous
"""Trainium2 Bass kernel for Conv2D(sum of 20 1x1 convs) + QwenRMSNorm.

Math: y = einsum("bsi,loi->bso", x, conv_w) / L ; out = rmsnorm(y) * norm_w.
Since x does not depend on l, the 20-matrix contraction collapses to a single
matmul with W = sum_l conv_w[l] / L.  Host pre-sums/transposes/casts the weight
(one [H,H] matrix bf16) and lays out x as token-sharded, hidden-major bf16
slabs; the 8 NeuronCores each run matmul (bf16, fp32 accum) + RMSNorm on their
2048 tokens.  All device compute is token-local; no collectives.

v3 schedule (trace-driven rework of v2):
 - MMs are ib-outer over GROUPS of 2 token tiles (4 PSUM bufs = all 8 banks),
   so real matmuls start as soon as x(tile0,1 lo-half) + w[ib0] land (~9.7us)
   instead of waiting for the whole 2MB weight fill (~16.9us in the v2 trace).
 - Input DMA descriptors are issued on the sync HWDGE ring in exact
   consumption order: x0/x1 lo, w0..w4, x0/x1 hi, w5..w7, x bulk.  One
   descriptor is >=128KB so the ~650ns/issue engine cadence never starves
   the ~390GB/s queue.
 - Warm-up MMs are N=128 on a small memset tile (fast cold dispatch) so the
   PE HAM clock-gate opens ~when the first real MM is data-ready.
 - Norm chain per tile: ACT square+accum -> ACT sqrt -> DVE recip, then the
   two output halves are scaled in PARALLEL (ACT Copy*scale on half1, DVE
   tensor_scalar on half0) to cut chain latency; one 256KB out DMA per tile,
   alternating scalar/gpsimd rings.
 - LAST tile runs oh1-first so its half1 square + sqrt-bias prep happen under
   the oh0 MMs; after the last MM only square(half0) -> sqrt -> recip ->
   parallel scales -> two half DMAs (sync+scalar HWDGE) remain (~4.4us tail
   vs 5.9us in v2).
 - norm_w == 1 (the spec's fill) skips the norm_w multiply; a general variant
   handles arbitrary norm_w.
"""

import numpy as np
import ml_dtypes
from contextlib import ExitStack

import concourse.bass as bass
import concourse.mybir as mybir
import concourse.tile as tile
from concourse.bass_utils import run_bass_kernel_spmd

N_CORES = 8
B, S, H, L = 4, 4096, 1024, 20
TOK = B * S               # 16384 tokens
TPC = TOK // N_CORES      # 2048 tokens per core
TB = TPC // 128           # 16 token-blocks of 128 per core
KB = H // 128             # 8 contraction blocks
NOH = H // 512            # 2 psum halves of the output row
EPS = 1e-6
N_WARM = 20               # HAM warm-up matmuls (N=128) during the DMA fill

BF16 = mybir.dt.bfloat16
F32 = mybir.dt.float32
AF = mybir.ActivationFunctionType
OP = mybir.AluOpType

_BUILT = {}          # variant -> cached Bass program
LAST_RESULTS = None  # BassKernelResults of the most recent run (for test harness)


def _legalize_multiwait(nc):
    """The walrus build here encodes exactly one semaphore wait per 64B
    instruction (NEURON_ISA_TPB_EVENTS has a single wait slot) and errors on
    Tile's multi-wait instructions.  Split surplus waits into standalone
    EVENT_SEMAPHORE instructions on the same engine, placed directly before
    the original instruction (same sequencer stream -> same semantics)."""
    n_ev = 0
    for f in nc.m.functions:
        for blk in f.blocks:
            insts = blk.instructions
            out = []
            changed = False
            for inst in list(insts):
                si = getattr(inst, "sync_info", None)
                waits = list(si.on_wait) if si is not None else []
                if len(waits) > 1:
                    changed = True
                    updates = list(si.on_update)
                    for w in waits[:-1]:
                        ev = mybir.InstEventSemaphore(
                            name=f"{inst.name}-sw{n_ev}", ins=[], outs=[])
                        n_ev += 1
                        ev.engine = inst.engine
                        ev.sync_info = mybir.SyncInfo(on_wait=[w], on_update=[])
                        out.append(ev)
                    inst.sync_info = mybir.SyncInfo(
                        on_wait=[waits[-1]], on_update=updates)
                out.append(inst)
            if changed:
                insts.clear()
                insts.extend(out)


def _build(with_nw):
    nc = bass.Bass()
    # x layout (partition-outermost): xt[p, tt, ib, t] = x[tt*128+t, ib*128+p]
    xt_h = nc.dram_tensor("xt", [128, TB, KB, 128], BF16, kind="ExternalInput")
    # w layout: wt[p, ib, oh, j] = W[oh*512+j, ib*128+p], W = sum_l conv_w[l]/L
    wt_h = nc.dram_tensor("wt", [128, KB, NOH, 512], BF16,
                          kind="ExternalInput")
    if with_nw:
        nw_h = nc.dram_tensor("nw", [H], F32, kind="ExternalInput")
    out_h = nc.dram_tensor("out", [TPC, H], BF16, kind="ExternalOutput")

    with tile.TileContext(nc) as tc, ExitStack() as ctx:
        xpool = ctx.enter_context(tc.tile_pool(name="x", bufs=1))
        wpool = ctx.enter_context(tc.tile_pool(name="w", bufs=1))
        cpool = ctx.enter_context(tc.tile_pool(name="consts", bufs=1))
        opool = ctx.enter_context(tc.tile_pool(name="out", bufs=6))
        spool = ctx.enter_context(tc.tile_pool(name="scratch", bufs=2))
        stats = ctx.enter_context(tc.tile_pool(name="stats", bufs=8))
        psum = ctx.enter_context(tc.tile_pool(name="psum", bufs=4, space="PSUM"))

        # const tiles first so the warm-up matmuls' source is ready early
        wu_sb = cpool.tile([128, 128], BF16)
        nc.vector.memset(wu_sb, 0.0)
        zero_sb = cpool.tile([128, 1], F32)
        nc.vector.memset(zero_sb, 0.0)
        eps_sb = cpool.tile([128, 1], F32)
        nc.vector.memset(eps_sb, EPS)

        x_sb = xpool.tile([128, TB, KB, 128], BF16)
        w_sb = wpool.tile([128, KB, NOH, 512], BF16)

        # Input DMA, sync HWDGE ring, in exact consumption order.  Descriptors
        # are >=128KB so the per-issue engine cadence (~650ns) keeps the queue
        # saturated at the ~390GB/s large-transfer rate.
        nc.sync.dma_start(out=x_sb[:, 0, 0:4], in_=xt_h[:, 0, 0:4])
        nc.sync.dma_start(out=x_sb[:, 1, 0:4], in_=xt_h[:, 1, 0:4])
        for ib in range(4):                                   # w0..w3
            nc.sync.dma_start(out=w_sb[:, ib:ib + 1], in_=wt_h[:, ib:ib + 1])
        nc.sync.dma_start(out=x_sb[:, 0, 4:8], in_=xt_h[:, 0, 4:8])
        nc.sync.dma_start(out=x_sb[:, 1, 4:8], in_=xt_h[:, 1, 4:8])
        for ib in range(4, KB):                               # w4..w7
            nc.sync.dma_start(out=w_sb[:, ib:ib + 1], in_=wt_h[:, ib:ib + 1])
        nc.sync.dma_start(out=x_sb[:, 2:4], in_=xt_h[:, 2:4])
        nc.sync.dma_start(out=x_sb[:, 4:8], in_=xt_h[:, 4:8])
        nc.sync.dma_start(out=x_sb[:, 8:16], in_=xt_h[:, 8:16])

        if with_nw:
            # norm_w broadcast to 128 partitions (general path only)
            nw_sb = cpool.tile([128, H], F32)
            nc.gpsimd.dma_start(
                out=nw_sb,
                in_=bass.AP(tensor=nw_h, offset=0, ap=[[0, 128], [1, H]]))

        # preload the ACT function table (Square/Sqrt) during the fill
        dummy = stats.tile([128, 1], F32)
        nc.scalar.activation(out=dummy, in_=zero_sb, func=AF.Square,
                             bias=zero_sb)

        # HAM warm-up: small N=128 matmuls dispatch quickly even at the cold
        # 1.2GHz clock, keeping the PE busy from ~7us so the HAM clock gate
        # is at 8/8 about when the first real matmul's data lands.
        wp = psum.tile([128, H], F32, name="wp", tag="yp")
        for _ in range(N_WARM):
            nc.tensor.matmul(wp[:, 0:128], wu_sb, wu_sb, start=True, stop=True)

        def norm_chain(tt, yp, last=False):
            """RMSNorm + scaled output + out-DMA for one 128-token tile."""
            rows = slice(tt * 128, (tt + 1) * 128)
            o_sb = opool.tile([128, H], BF16)
            if last:
                # halves were squared separately (oh1 early, under oh0 MMs)
                sq1 = spool.tile([128, 512], BF16, tag="sq")
                ssum1 = stats.tile([128, 1], F32)
                nc.scalar.activation(out=sq1, in_=yp[:, 512:1024],
                                     func=AF.Square, bias=zero_sb,
                                     accum_out=ssum1)
                # bias-prep for the final sqrt: b = ssum1/H + eps
                bprep = stats.tile([128, 1], F32)
                nc.scalar.activation(out=bprep, in_=ssum1, func=AF.Identity,
                                     bias=eps_sb, scale=1.0 / H)
                yield  # emit oh0 MMs in the caller, then resume
                sq0 = spool.tile([128, 512], BF16, tag="sq")
                ssum0 = stats.tile([128, 1], F32)
                nc.scalar.activation(out=sq0, in_=yp[:, 0:512],
                                     func=AF.Square, bias=zero_sb,
                                     accum_out=ssum0)
                std = stats.tile([128, 1], F32)
                nc.scalar.activation(out=std, in_=ssum0, func=AF.Sqrt,
                                     bias=bprep, scale=1.0 / H)
            else:
                sq = spool.tile([128, H], BF16, tag="sq")
                ssum = stats.tile([128, 1], F32)
                nc.scalar.activation(out=sq, in_=yp, func=AF.Square,
                                     bias=zero_sb, accum_out=ssum)
                std = stats.tile([128, 1], F32)
                nc.scalar.activation(out=std, in_=ssum, func=AF.Sqrt,
                                     bias=eps_sb, scale=1.0 / H)
            rstd = stats.tile([128, 1], F32)
            nc.vector.reciprocal(out=rstd, in_=std)
            # scale the two halves in parallel: half1 on ACT, half0 on DVE
            if with_nw:
                nc.vector.scalar_tensor_tensor(
                    out=o_sb[:, 512:1024], in0=yp[:, 512:1024], scalar=rstd,
                    in1=nw_sb[:, 512:1024], op0=OP.mult, op1=OP.mult)
                nc.vector.scalar_tensor_tensor(
                    out=o_sb[:, 0:512], in0=yp[:, 0:512], scalar=rstd,
                    in1=nw_sb[:, 0:512], op0=OP.mult, op1=OP.mult)
            else:
                nc.scalar.activation(out=o_sb[:, 512:1024],
                                     in_=yp[:, 512:1024], func=AF.Copy,
                                     scale=rstd)
                nc.vector.tensor_scalar_mul(out=o_sb[:, 0:512],
                                            in0=yp[:, 0:512], scalar1=rstd)
            if last:
                # two half DMAs on the two idle HWDGE rings
                nc.scalar.dma_start(out=out_h[rows, 512:1024],
                                    in_=o_sb[:, 512:1024])
                nc.sync.dma_start(out=out_h[rows, 0:512],
                                  in_=o_sb[:, 0:512])
            else:
                # one 256KB contiguous DMA; alternate rings per tile.  SWDGE
                # (gpsimd) only for early tiles so its queue is long drained
                # by tile-context exit.
                eng = nc.gpsimd if (tt % 2 == 0 and tt < 13) else nc.scalar
                eng.dma_start(out=out_h[rows, :], in_=o_sb)

        # groups of 2 token tiles, ib-outer inside the group: matmuls start
        # once x(lo)+w[ib0] are resident and stay DMA-fed at ~300GB/s.
        groups = [(0, 1), (2, 3), (4, 5), (6, 7), (8, 9), (10, 11), (12, 13),
                  (14,), (15,)]
        for tts in groups:
            yps = [psum.tile([128, H], F32, name=f"yp{tt}", tag="yp")
                   for tt in tts]
            if tts[-1] == TB - 1:
                # last tile: oh1 MMs first so its square/bias-prep run under
                # the oh0 MMs; only the short half0 chain trails the last MM.
                tt = tts[0]
                yp = yps[0]
                gen = norm_chain(tt, yp, last=True)
                for oh in (1, 0):
                    for ib in range(KB):
                        nc.tensor.matmul(
                            yp[:, oh * 512:(oh + 1) * 512],
                            x_sb[:, tt, ib, :],
                            w_sb[:, ib, oh, :],
                            start=(ib == 0), stop=(ib == KB - 1))
                    if oh == 1:
                        next(gen)       # emit early half1 square + bias prep
                for _ in gen:           # emit the trailing chain
                    pass
            else:
                # ib-half blocks: 4 consecutive MMs accumulate into the SAME
                # psum bank (per-MM bank switching costs ~50ns of PE bubble,
                # measured), while the half granularity still lets compute
                # start on the first half of the weight fill.
                for ibh in range(2):
                    for i, tt in enumerate(tts):
                        for oh in range(NOH):
                            for ib in range(ibh * 4, ibh * 4 + 4):
                                nc.tensor.matmul(
                                    yps[i][:, oh * 512:(oh + 1) * 512],
                                    x_sb[:, tt, ib, :],
                                    w_sb[:, ib, oh, :],
                                    start=(ib == 0), stop=(ib == KB - 1))
                for i, tt in enumerate(tts):
                    for _ in norm_chain(tt, yps[i]):
                        pass

    _legalize_multiwait(nc)
    return nc


def host_prep(x, conv_w, norm_w, with_nw):
    """Shard + lay out the full inputs into per-core device input maps."""
    bf16 = ml_dtypes.bfloat16

    # Collapse the 20 1x1 convs: W[o,i] = sum_l conv_w[l,o,i] / L
    w = np.asarray(conv_w).sum(axis=0) * (1.0 / L)          # [H(o), H(i)] f32
    # wt[p, ib, oh, j] = W[oh*512+j, ib*128+p]
    wt = np.ascontiguousarray(
        w.reshape(NOH, 512, KB, 128).transpose(3, 2, 0, 1).astype(bf16))

    x2d = np.asarray(x).reshape(TOK, H)
    xbf = x2d.astype(bf16)

    in_maps = []
    for c in range(N_CORES):
        xc = xbf[c * TPC:(c + 1) * TPC]                      # [TPC, H]
        # xt[p, tt, ib, t] = xc[tt*128+t, ib*128+p]
        xtc = np.ascontiguousarray(
            xc.reshape(TB, 128, KB, 128).transpose(3, 0, 2, 1))
        m = {"xt": xtc, "wt": wt}
        if with_nw:
            m["nw"] = np.ascontiguousarray(np.asarray(norm_w),
                                           dtype=np.float32)
        in_maps.append(m)
    return in_maps


def kernel(x, conv_w, norm_w):
    global LAST_RESULTS

    x = np.asarray(x)
    out_dtype = x.dtype
    nw = np.asarray(norm_w)
    with_nw = not bool(np.all(nw == 1.0))

    if with_nw not in _BUILT:
        _BUILT[with_nw] = _build(with_nw)
    nc = _BUILT[with_nw]

    in_maps = host_prep(x, conv_w, norm_w, with_nw)

    res = run_bass_kernel_spmd(nc, in_maps, core_ids=list(range(N_CORES)))
    LAST_RESULTS = res

    out = np.concatenate([r["out"] for r in res.results], axis=0)
    return out.reshape(B, S, H).astype(out_dtype, copy=False)
